# revision 24
# baseline (speedup 1.0000x reference)
"""Distributed Trainium2 (Bass/Tile) kernel for the contrastive loss.

Ring-partitioned symmetric-similarity scheme (8 NeuronCores, SPMD):

  Global per l: 4096 rows in 32 chunks of 128.  sim = Z Z^T is symmetric,
  so each unordered chunk pair is computed ONCE: chunk i covers column
  chunks {i..i+15} (ring-forward), and the distance-16 block is computed
  by both endpoints with its exp HALVED (bias ln 1/2).  Core c owns row
  chunks {4c..4c+3}; it therefore only needs Z chunks {4c..4c+19} (a
  20-chunk window, rolled so the window is local chunks 0..19).

  Per core:
    - load raw window rows in natural layout [128, l, 20, 128] (fp32),
    - ssq via fused square+row-sum on DVE; invn = exp(-0.5 ln ssq) (ACT),
    - scale rows by invn -> bf16 zb (GpSimd tensor_scalar),
    - transpose zb via the XBAR DMA-transpose -> xt [d, chunk, row] bf16,
      (prep chain runs in quarters so the first matmul starts early),
    - row-chunk i: 4x 512-col bf16 matmuls -> PSUM [128, 2048];
      ACT exp (scale 1/T) -> E bf16 SBUF + per-row accum (denominator
      row part);  per covered column chunk a 1-col ones-matmul on PE
      computes the column sums (the denominator part owed to OTHER
      row chunks); distance-16 block separately with bias ln(1/2),
    - positive pairs are exactly the distance-16 diagonals:
      pos = zb[:,i,:] . zb[:,i+16,:] row-dots on DVE.
  Outputs per core (one packed DMA): row accums, d16 row sums, pos dots
  and column-sum partials.  The host does the tiny cross-core assembly:
  denominators = row part + mapped column partials - e^5 (self term),
  then loss = sum w (-pos/T + log denom) / (2 sum w).
"""

import numpy as np

TEMP = 0.2
INV_T = 1.0 / TEMP
L, B, K, D = 4, 64, 32, 128
N = B * K            # 2048
M = 2 * N            # 4096 rows per l
NCH = 32             # global 128-row chunks per l
NCORES = 8
RC = 4               # row chunks owned per core
W = 20               # chunk window per core (rc spans + d16 partners)
SPAN = 16            # forward span chunks (excl. the halved d16 block)
QS = 5               # prep quarter size (chunks per quarter)

_built = None


def _build():
    global _built
    if _built is not None:
        return _built
    from contextlib import ExitStack

    import concourse.tile as tile
    from concourse import bacc
    import concourse.mybir as mybir

    f32 = mybir.dt.float32
    bf16 = mybir.dt.bfloat16
    AF = mybir.ActivationFunctionType
    OP = mybir.AluOpType
    AX = mybir.AxisListType

    # Pin every ACT op to the natural_log_exp_and_others table set (covers
    # Copy/Exp/Identity/Ln) so bacc emits exactly one LoadActFuncSet.
    from concourse import hw_specs as _hw
    _tabs = dict(_hw.get_activation_tables("gen3"))
    _pinned = {
        name: (fns if name == "natural_log_exp_and_others" else frozenset())
        for name, fns in _tabs.items()
    }
    _hw.get_activation_tables.cache_clear()
    _orig = _hw.get_activation_tables.__wrapped__

    def _patched(arch):
        if arch == "gen3":
            return _pinned
        return _orig(arch)

    _hw.get_activation_tables = _patched
    import concourse.bacc as _baccmod
    if hasattr(_baccmod, "get_activation_tables"):
        _baccmod.get_activation_tables = _patched

    nc = bacc.Bacc(None, target_bir_lowering=False)
    emb = nc.dram_tensor("emb_nat", [128, L, W, D], f32, kind="ExternalInput")
    # per l: [dsum(4) | dsplit(1) | d16r(4) | pos(4) | cacc(20)] = 33 cols
    PL = 33
    out = nc.dram_tensor("out_all", [128, L, PL], f32, kind="ExternalOutput")

    with ExitStack() as ctx:
        tc = ctx.enter_context(tile.TileContext(nc))
        singles = ctx.enter_context(tc.tile_pool(name="singles", bufs=1))
        natp = ctx.enter_context(tc.tile_pool(name="nat", bufs=4))
        zbp = ctx.enter_context(tc.tile_pool(name="zb", bufs=3))
        xtp = ctx.enter_context(tc.tile_pool(name="xt", bufs=3))
        statp = ctx.enter_context(tc.tile_pool(name="stat", bufs=3))
        ep = ctx.enter_context(tc.tile_pool(name="eo", bufs=3))
        e16p = ctx.enter_context(tc.tile_pool(name="e16", bufs=2))
        simp = ctx.enter_context(tc.tile_pool(name="sim", bufs=2, space="PSUM"))

        ones_bf = singles.tile([128, 1], bf16)
        nc.vector.memset(ones_bf[:], 1.0)
        ln_half = singles.tile([128, 1], f32)
        nc.vector.memset(ln_half[:], float(np.log(0.5)))
        acc = singles.tile([128, L, PL], f32)
        nc.vector.memset(acc[:], 0.0)

        def dsum(l, i):
            return acc[:, l, i : i + 1]

        def d16r_sl(l):
            return acc[:, l, 5:9]

        def pos_sl(l, i):
            return acc[:, l, 9 + i : 10 + i]

        def cacc_sl(l, j0, j1):
            return acc[:, l, 13 + j0 : 13 + j1]

        junk = singles.tile([128, D], f32)
        junkb = singles.tile([128, D], bf16)

        nats, zbs, xts = {}, {}, {}

        def load(l):
            # quartered input DMAs so XBAR transposes can interleave on
            # the (exclusive) DMA-engine device
            nat = natp.tile([128, W, D], f32, tag="nat", name=f"nat{l}")
            for q0 in range(0, W, QS):
                nc.sync.dma_start(
                    out=nat[:, q0 : q0 + QS, :], in_=emb[:, l, q0 : q0 + QS, :])
            nats[l] = nat

        def prep_tiles(l):
            ssq = statp.tile([128, W], f32, tag="ssq", name=f"ssq{l}")
            lnssq = statp.tile([128, W], f32, tag="lnssq", name=f"lnssq{l}")
            invn = statp.tile([128, W], f32, tag="invn", name=f"invn{l}")
            zb = zbp.tile([128, W, D], bf16, tag="zb", name=f"zb{l}")
            xt = xtp.tile([128, W, 128], bf16, tag="xt", name=f"xt{l}")
            zbs[l], xts[l] = zb, xt
            return ssq, lnssq, invn, zb, xt

        def prep_ssq(l, st, q0, q1):
            ssq = st[0]
            for s in range(q0, q1):
                nc.vector.scalar_tensor_tensor(
                    out=junk[:], in0=nats[l][:, s, :], scalar=1.0,
                    in1=nats[l][:, s, :], op0=OP.mult, op1=OP.mult,
                    accum_out=ssq[:, s : s + 1])

        def prep_rest(l, st, q0, q1):
            ssq, lnssq, invn, zb, xt = st
            nc.scalar.activation(out=lnssq[:, q0:q1], in_=ssq[:, q0:q1],
                                 func=AF.Ln)
            nc.scalar.activation(out=invn[:, q0:q1], in_=lnssq[:, q0:q1],
                                 func=AF.Exp, scale=-0.5)
            for s in range(q0, q1):
                nc.gpsimd.tensor_scalar_mul(
                    zb[:, s, :], nats[l][:, s, :], invn[:, s : s + 1])
            zbf = zb[:].rearrange("p s d -> p (s d)")
            nc.sync.dma_start_transpose(
                out=xt[:, q0:q1, :], in_=zbf[:, q0 * D : q1 * D])

        def span_mm(l, i, st, c_lo=0, c_hi=SPAN * 128):
            xt = xts[l]
            xtf = xt[:].rearrange("p s r -> p (s r)")
            for q0 in range(c_lo, c_hi, 512):
                q1 = min(q0 + 512, c_hi)
                nc.tensor.matmul(
                    st[:, q0:q1], xt[:, i, :],
                    xtf[:, i * 128 + q0 : i * 128 + q1], start=True, stop=True)

        def span_exp(l, i, st, accum, c_lo=0, c_hi=SPAN * 128, e=None):
            if e is None:
                e = ep.tile([128, SPAN * 128], bf16, tag="e", name=f"e{l}_{i}")
            nc.scalar.activation(
                out=e[:, c_lo:c_hi], in_=st[:, c_lo:c_hi], func=AF.Exp,
                scale=INV_T, accum_out=accum)
            return e

        def span_colsum(l, i, st, e):
            for k in range(1, SPAN):
                nc.tensor.matmul(
                    st[:, k - 1 : k], e[:, k * 128 : (k + 1) * 128],
                    ones_bf[:], start=True, stop=True)
            nc.vector.tensor_tensor(
                out=cacc_sl(l, i + 1, i + SPAN),
                in0=cacc_sl(l, i + 1, i + SPAN),
                in1=st[:, 0 : SPAN - 1], op=OP.add)

        def d16_mm(l, st2):
            xt = xts[l]
            for i in range(RC):
                nc.tensor.matmul(
                    st2[:, i * 128 : (i + 1) * 128], xt[:, i, :],
                    xt[:, i + SPAN, :], start=True, stop=True)

        def d16_exp_cs(l, st2):
            e16 = e16p.tile([128, RC, 128], bf16, tag="e16", name=f"e16_{l}")
            nc.scalar.activation(
                out=e16[:].rearrange("p a b -> p (a b)"), in_=st2[:, 0:512],
                func=AF.Exp, scale=INV_T, bias=ln_half[:])
            for i in range(RC):
                nc.tensor.matmul(
                    st2[:, 512 + i : 513 + i], e16[:, i, :], ones_bf[:],
                    start=True, stop=True)
            nc.vector.reduce_sum(
                out=d16r_sl(l), in_=e16[:], axis=AX.X)
            nc.vector.tensor_tensor(
                out=cacc_sl(l, SPAN, SPAN + RC),
                in0=cacc_sl(l, SPAN, SPAN + RC),
                in1=st2[:, 512 : 512 + RC], op=OP.add)

        def pos_dots(l):
            zb = zbs[l]
            for i in range(RC):
                nc.vector.scalar_tensor_tensor(
                    out=junkb[:], in0=zb[:, i, :], scalar=1.0,
                    in1=zb[:, i + SPAN, :], op0=OP.mult, op1=OP.mult,
                    accum_out=pos_sl(l, i))

        def newst(l, i):
            return simp.tile([128, SPAN * 128], f32, tag="st",
                             name=f"st{l}_{i}")

        def prep_next(l):
            # finish l's normalization chain: invn (ACT) -> scale (Pool)
            # -> XBAR transpose (SP/DMA), emitted at a point where the
            # ACT queue ahead of it is already drained
            prep_rest(l, preps[l], 0, W)

        def sims0():
            # l=0: rc0's span exp split so ACT starts as soon as the
            # first two prep quarters (chunks 0..9) are transposed.
            # Tile allocation strictly at first use: a buffer's next
            # writer must be emitted after every access to its previous
            # generation (clean WAR ordering for the 2-deep PSUM ring).
            l = 0
            st0 = newst(l, 0)
            span_mm(l, 0, st0, 0, 1024)
            e0 = span_exp(l, 0, st0, dsum(l, 0), 0, 1024)
            span_mm(l, 0, st0, 1024, 2048)
            span_exp(l, 0, st0, acc[:, 0, 4:5], 1024, 2048, e=e0)
            load(2)
            prep_next(1)
            st1 = newst(l, 1)
            span_mm(l, 1, st1)
            e1 = span_exp(l, 1, st1, dsum(l, 1))
            span_colsum(l, 0, st0, e0)
            st2 = simp.tile([128, SPAN * 128], f32, tag="st", name="d16st0")
            d16_mm(l, st2)
            d16_exp_cs(l, st2)
            span_colsum(l, 1, st1, e1)
            preps[2] = prep_tiles(2)
            prep_ssq(2, preps[2], 0, W)
            st3 = newst(l, 2)
            span_mm(l, 2, st3)
            e2 = span_exp(l, 2, st3, dsum(l, 2))
            st4 = newst(l, 3)
            span_mm(l, 3, st4)
            e3 = span_exp(l, 3, st4, dsum(l, 3))
            prep_next(2)
            span_colsum(l, 2, st3, e2)
            span_colsum(l, 3, st4, e3)
            pos_dots(l)
            nc.sync.dma_start(out=out[:, l, :], in_=acc[:, l, :])

        def sims(l):
            # steady state: d16 first (its tiny matmuls + exp fill the
            # ACT bubble at the l-transition while span buffers drain)
            if l + 2 < L:
                load(l + 2)
            st2 = simp.tile([128, SPAN * 128], f32, tag="st", name=f"d16st{l}")
            d16_mm(l, st2)
            st0 = newst(l, 0)
            span_mm(l, 0, st0)
            d16_exp_cs(l, st2)
            e0 = span_exp(l, 0, st0, dsum(l, 0))
            st1 = newst(l, 1)
            span_mm(l, 1, st1)
            e1 = span_exp(l, 1, st1, dsum(l, 1))
            span_colsum(l, 0, st0, e0)
            st2b = newst(l, 2)
            span_mm(l, 2, st2b)
            e2 = span_exp(l, 2, st2b, dsum(l, 2))
            span_colsum(l, 1, st1, e1)
            if l + 2 < L:
                preps[l + 2] = prep_tiles(l + 2)
                prep_ssq(l + 2, preps[l + 2], 0, W)
            st3 = newst(l, 3)
            span_mm(l, 3, st3)
            e3 = span_exp(l, 3, st3, dsum(l, 3))
            if l + 2 < L:
                prep_next(l + 2)
            span_colsum(l, 2, st2b, e2)
            span_colsum(l, 3, st3, e3)
            pos_dots(l)
            nc.sync.dma_start(out=out[:, l, :], in_=acc[:, l, :])

        preps = {}
        load(0)
        load(1)
        preps[0] = prep_tiles(0)
        for q0 in range(0, W, QS):
            prep_ssq(0, preps[0], q0, q0 + QS)
            prep_rest(0, preps[0], q0, q0 + QS)
        preps[1] = prep_tiles(1)
        prep_ssq(1, preps[1], 0, W)
        sims0()
        sims(1)
        sims(2)
        sims(3)

    nc.finalize()
    _built = nc
    return nc


def _in_maps(emb_i, emb_j, joint_valid):
    emb_i = np.asarray(emb_i, dtype=np.float32)
    emb_j = np.asarray(emb_j, dtype=np.float32)
    jv = np.asarray(joint_valid, dtype=np.float32).reshape(-1)
    reps = np.concatenate(
        [emb_i.reshape(L, N, D), emb_j.reshape(L, N, D)], axis=1)  # [L, M, D]
    repsc = reps.reshape(L, NCH, 128, D)
    maps = []
    for c in range(NCORES):
        sel = (RC * c + np.arange(W)) % NCH
        win = repsc[:, sel]                       # [L, W, 128, D]
        nat = np.ascontiguousarray(win.transpose(2, 0, 1, 3))  # [128, L, W, D]
        maps.append({"emb_nat": nat})
    return maps, jv


def _combine(results, jv):
    E5 = float(np.exp(INV_T))  # self-similarity exp(1/T)
    denom = np.zeros((L, NCH, 128), dtype=np.float64)
    posg = np.zeros((L, NCH, 128), dtype=np.float64)
    for c, r in enumerate(results):
        a = r["out_all"].astype(np.float64)     # [128, L, 33]
        for l in range(L):
            for i in range(RC):
                g = (RC * c + i) % NCH
                denom[l, g] += a[:, l, i] + a[:, l, 5 + i]
                posg[l, g] = a[:, l, 9 + i]
            denom[l, RC * c % NCH] += a[:, l, 4]  # l0-rc0 split piece B
            for j in range(1, W):
                g = (RC * c + j) % NCH
                denom[l, g] += a[:, l, 13 + j]
    denom -= E5
    w = jv.astype(np.float64)                   # [N]
    wrow = np.concatenate([w, w]).reshape(NCH, 128)  # weight per global row
    lp = -posg * INV_T + np.log(denom)          # [L, NCH, 128]
    loss = (lp * wrow[None]).sum() / (2.0 * w.sum())
    return np.float32(loss)


def kernel(emb_i, emb_j, joint_valid):
    from concourse.bass_utils import run_bass_kernel_spmd

    nc = _build()
    maps, jv = _in_maps(emb_i, emb_j, joint_valid)
    res = run_bass_kernel_spmd(nc, maps, core_ids=list(range(NCORES)))
    return _combine(res.results, jv)


def run_traced(inputs, trace_cores=None):
    """test.py helper: same run but with NTFF tracing enabled."""
    from concourse.bass_utils import run_bass_kernel_spmd

    nc = _build()
    maps, jv = _in_maps(**inputs)
    res = run_bass_kernel_spmd(
        nc, maps, core_ids=list(range(NCORES)), trace=True,
        trace_cores=trace_cores if trace_cores is not None else list(range(NCORES)))
    res.loss = _combine(res.results, jv)
    return res


# revision 25
# speedup vs baseline: 1.1026x; 1.1026x over previous
"""Distributed Trainium2 (Bass/Tile) kernel for the contrastive loss.

Ring-partitioned symmetric-similarity scheme (8 NeuronCores, SPMD):

  Global per l: 4096 rows in 32 chunks of 128.  sim = Z Z^T is symmetric,
  so each unordered chunk pair is computed ONCE: chunk i covers column
  chunks {i..i+15} (ring-forward), and the distance-16 block is computed
  by both endpoints with its exp HALVED (bias ln 1/2).  Core c owns row
  chunks {4c..4c+3}; it therefore only needs Z chunks {4c..4c+19} (a
  20-chunk window, rolled so the window is local chunks 0..19).

  Per core:
    - load raw window rows in natural layout [128, l, 20, 128] (fp32),
    - ssq via fused square+row-sum on DVE; invn = exp(-0.5 ln ssq) (ACT),
    - scale rows by invn -> bf16 zb (GpSimd tensor_scalar),
    - transpose zb via the XBAR DMA-transpose -> xt [d, chunk, row] bf16,
      (prep chain runs in quarters so the first matmul starts early),
    - row-chunk i: 4x 512-col bf16 matmuls -> PSUM [128, 2048];
      ACT exp (scale 1/T) -> E bf16 SBUF + per-row accum (denominator
      row part);  per covered column chunk a 1-col ones-matmul on PE
      computes the column sums (the denominator part owed to OTHER
      row chunks); distance-16 block separately with bias ln(1/2),
    - positive pairs are exactly the distance-16 diagonals:
      pos = zb[:,i,:] . zb[:,i+16,:] row-dots on DVE.
  Outputs per core (one packed DMA): row accums, d16 row sums, pos dots
  and column-sum partials.  The host does the tiny cross-core assembly:
  denominators = row part + mapped column partials - e^5 (self term),
  then loss = sum w (-pos/T + log denom) / (2 sum w).
"""

import numpy as np

TEMP = 0.2
INV_T = 1.0 / TEMP
L, B, K, D = 4, 64, 32, 128
N = B * K            # 2048
M = 2 * N            # 4096 rows per l
NCH = 32             # global 128-row chunks per l
NCORES = 8
RC = 4               # row chunks owned per core
W = 20               # chunk window per core (rc spans + d16 partners)
SPAN = 16            # forward span chunks (excl. the halved d16 block)
QS = 5               # prep quarter size (chunks per quarter)

_built = None


def _build():
    global _built
    if _built is not None:
        return _built
    from contextlib import ExitStack

    import concourse.tile as tile
    from concourse import bacc
    import concourse.mybir as mybir

    f32 = mybir.dt.float32
    bf16 = mybir.dt.bfloat16
    AF = mybir.ActivationFunctionType
    OP = mybir.AluOpType
    AX = mybir.AxisListType

    # Pin every ACT op to the natural_log_exp_and_others table set (covers
    # Copy/Exp/Identity/Ln) so bacc emits exactly one LoadActFuncSet.
    from concourse import hw_specs as _hw
    _tabs = dict(_hw.get_activation_tables("gen3"))
    _pinned = {
        name: (fns if name == "natural_log_exp_and_others" else frozenset())
        for name, fns in _tabs.items()
    }
    _hw.get_activation_tables.cache_clear()
    _orig = _hw.get_activation_tables.__wrapped__

    def _patched(arch):
        if arch == "gen3":
            return _pinned
        return _orig(arch)

    _hw.get_activation_tables = _patched
    import concourse.bacc as _baccmod
    if hasattr(_baccmod, "get_activation_tables"):
        _baccmod.get_activation_tables = _patched

    nc = bacc.Bacc(None, target_bir_lowering=False)
    emb = nc.dram_tensor("emb_nat", [128, L, W, D], f32, kind="ExternalInput")
    # per l: [dsum(4) | dsplit(1) | d16r(4) | pos(4) | cacc(20)] = 33 cols
    PL = 33
    out = nc.dram_tensor("out_all", [128, L, PL], f32, kind="ExternalOutput")

    with ExitStack() as ctx:
        tc = ctx.enter_context(tile.TileContext(nc))
        singles = ctx.enter_context(tc.tile_pool(name="singles", bufs=1))
        natp = ctx.enter_context(tc.tile_pool(name="nat", bufs=4))
        zbp = ctx.enter_context(tc.tile_pool(name="zb", bufs=3))
        xtp = ctx.enter_context(tc.tile_pool(name="xt", bufs=3))
        statp = ctx.enter_context(tc.tile_pool(name="stat", bufs=3))
        ep = ctx.enter_context(tc.tile_pool(name="eo", bufs=3))
        e16p = ctx.enter_context(tc.tile_pool(name="e16", bufs=2))
        simp = ctx.enter_context(tc.tile_pool(name="sim", bufs=2, space="PSUM"))

        ones_bf = singles.tile([128, 1], bf16)
        nc.vector.memset(ones_bf[:], 1.0)
        ln_half = singles.tile([128, 1], f32)
        nc.vector.memset(ln_half[:], float(np.log(0.5)))
        acc = singles.tile([128, L, PL], f32)
        nc.vector.memset(acc[:], 0.0)

        def dsum(l, i):
            return acc[:, l, i : i + 1]

        def d16r_sl(l):
            return acc[:, l, 5:9]

        def pos_sl(l, i):
            return acc[:, l, 9 + i : 10 + i]

        def cacc_sl(l, j0, j1):
            return acc[:, l, 13 + j0 : 13 + j1]

        junk = singles.tile([128, D], f32)
        junkb = singles.tile([128, D], bf16)

        nats, zbs, xts = {}, {}, {}

        def load(l):
            # quartered input DMAs so XBAR transposes can interleave on
            # the (exclusive) DMA-engine device
            nat = natp.tile([128, W, D], f32, tag="nat", name=f"nat{l}")
            for q0 in range(0, W, QS):
                nc.sync.dma_start(
                    out=nat[:, q0 : q0 + QS, :], in_=emb[:, l, q0 : q0 + QS, :])
            nats[l] = nat

        def prep_tiles(l):
            ssq = statp.tile([128, W], f32, tag="ssq", name=f"ssq{l}")
            lnssq = statp.tile([128, W], f32, tag="lnssq", name=f"lnssq{l}")
            invn = statp.tile([128, W], f32, tag="invn", name=f"invn{l}")
            zb = zbp.tile([128, W, D], bf16, tag="zb", name=f"zb{l}")
            xt = xtp.tile([128, W, 128], bf16, tag="xt", name=f"xt{l}")
            zbs[l], xts[l] = zb, xt
            return ssq, lnssq, invn, zb, xt

        def prep_ssq(l, st, q0, q1):
            ssq = st[0]
            for s in range(q0, q1):
                nc.vector.scalar_tensor_tensor(
                    out=junk[:], in0=nats[l][:, s, :], scalar=1.0,
                    in1=nats[l][:, s, :], op0=OP.mult, op1=OP.mult,
                    accum_out=ssq[:, s : s + 1])

        def prep_rest(l, st, q0, q1):
            # normalization chain is latency-critical for the next l's
            # matmuls: high_priority so the scheduler never parks it
            # behind exp work, and the scale pass is split Pool/DVE to
            # halve its serial length
            ssq, lnssq, invn, zb, xt = st
            with tc.high_priority():
                nc.scalar.activation(out=lnssq[:, q0:q1], in_=ssq[:, q0:q1],
                                     func=AF.Ln)
                nc.scalar.activation(out=invn[:, q0:q1], in_=lnssq[:, q0:q1],
                                     func=AF.Exp, scale=-0.5)
                h = (q0 + q1) // 2
                for s in range(q0, h):
                    nc.gpsimd.tensor_scalar_mul(
                        zb[:, s, :], nats[l][:, s, :], invn[:, s : s + 1])
                for s in range(h, q1):
                    nc.vector.tensor_scalar_mul(
                        zb[:, s, :], nats[l][:, s, :], invn[:, s : s + 1])
                zbf = zb[:].rearrange("p s d -> p (s d)")
                nc.sync.dma_start_transpose(
                    out=xt[:, q0:q1, :], in_=zbf[:, q0 * D : q1 * D])

        def span_mm(l, i, st, c_lo=0, c_hi=SPAN * 128):
            xt = xts[l]
            xtf = xt[:].rearrange("p s r -> p (s r)")
            for q0 in range(c_lo, c_hi, 512):
                q1 = min(q0 + 512, c_hi)
                nc.tensor.matmul(
                    st[:, q0:q1], xt[:, i, :],
                    xtf[:, i * 128 + q0 : i * 128 + q1], start=True, stop=True)

        def span_exp(l, i, st, accum, c_lo=0, c_hi=SPAN * 128, e=None):
            if e is None:
                e = ep.tile([128, SPAN * 128], bf16, tag="e", name=f"e{l}_{i}")
            nc.scalar.activation(
                out=e[:, c_lo:c_hi], in_=st[:, c_lo:c_hi], func=AF.Exp,
                scale=INV_T, accum_out=accum)
            return e

        def span_colsum(l, i, st, e):
            for k in range(1, SPAN):
                nc.tensor.matmul(
                    st[:, k - 1 : k], e[:, k * 128 : (k + 1) * 128],
                    ones_bf[:], start=True, stop=True)
            nc.vector.tensor_tensor(
                out=cacc_sl(l, i + 1, i + SPAN),
                in0=cacc_sl(l, i + 1, i + SPAN),
                in1=st[:, 0 : SPAN - 1], op=OP.add)

        def d16_mm(l, st2):
            xt = xts[l]
            for i in range(RC):
                nc.tensor.matmul(
                    st2[:, i * 128 : (i + 1) * 128], xt[:, i, :],
                    xt[:, i + SPAN, :], start=True, stop=True)

        def d16_exp_cs(l, st2):
            e16 = e16p.tile([128, RC, 128], bf16, tag="e16", name=f"e16_{l}")
            nc.scalar.activation(
                out=e16[:].rearrange("p a b -> p (a b)"), in_=st2[:, 0:512],
                func=AF.Exp, scale=INV_T, bias=ln_half[:])
            for i in range(RC):
                nc.tensor.matmul(
                    st2[:, 512 + i : 513 + i], e16[:, i, :], ones_bf[:],
                    start=True, stop=True)
            nc.vector.reduce_sum(
                out=d16r_sl(l), in_=e16[:], axis=AX.X)
            nc.vector.tensor_tensor(
                out=cacc_sl(l, SPAN, SPAN + RC),
                in0=cacc_sl(l, SPAN, SPAN + RC),
                in1=st2[:, 512 : 512 + RC], op=OP.add)

        def pos_dots(l):
            zb = zbs[l]
            for i in range(RC):
                nc.vector.scalar_tensor_tensor(
                    out=junkb[:], in0=zb[:, i, :], scalar=1.0,
                    in1=zb[:, i + SPAN, :], op0=OP.mult, op1=OP.mult,
                    accum_out=pos_sl(l, i))

        def newst(l, i):
            return simp.tile([128, SPAN * 128], f32, tag="st",
                             name=f"st{l}_{i}")

        def prep_next(l):
            # finish l's normalization chain: invn (ACT) -> scale (Pool)
            # -> XBAR transpose (SP/DMA), emitted at a point where the
            # ACT queue ahead of it is already drained
            prep_rest(l, preps[l], 0, W)

        def sims0():
            # l=0: rc0's span exp split so ACT starts as soon as the
            # first two prep quarters (chunks 0..9) are transposed.
            # Tile allocation strictly at first use: a buffer's next
            # writer must be emitted after every access to its previous
            # generation (clean WAR ordering for the 2-deep PSUM ring).
            l = 0
            st0 = newst(l, 0)
            span_mm(l, 0, st0, 0, 1024)
            e0 = span_exp(l, 0, st0, dsum(l, 0), 0, 1024)
            span_mm(l, 0, st0, 1024, 2048)
            span_exp(l, 0, st0, acc[:, 0, 4:5], 1024, 2048, e=e0)
            load(2)
            prep_next(1)
            st1 = newst(l, 1)
            span_mm(l, 1, st1)
            e1 = span_exp(l, 1, st1, dsum(l, 1))
            span_colsum(l, 0, st0, e0)
            st2 = simp.tile([128, SPAN * 128], f32, tag="st", name="d16st0")
            d16_mm(l, st2)
            d16_exp_cs(l, st2)
            span_colsum(l, 1, st1, e1)
            preps[2] = prep_tiles(2)
            prep_ssq(2, preps[2], 0, W)
            st3 = newst(l, 2)
            span_mm(l, 2, st3)
            e2 = span_exp(l, 2, st3, dsum(l, 2))
            st4 = newst(l, 3)
            span_mm(l, 3, st4)
            e3 = span_exp(l, 3, st4, dsum(l, 3))
            prep_next(2)
            span_colsum(l, 2, st3, e2)
            span_colsum(l, 3, st4, e3)
            pos_dots(l)
            nc.sync.dma_start(out=out[:, l, :], in_=acc[:, l, :])

        def sims(l):
            # steady state: d16 first (its tiny matmuls + exp fill the
            # ACT bubble at the l-transition while span buffers drain)
            if l + 2 < L:
                load(l + 2)
            st2 = simp.tile([128, SPAN * 128], f32, tag="st", name=f"d16st{l}")
            d16_mm(l, st2)
            st0 = newst(l, 0)
            span_mm(l, 0, st0)
            d16_exp_cs(l, st2)
            e0 = span_exp(l, 0, st0, dsum(l, 0))
            st1 = newst(l, 1)
            span_mm(l, 1, st1)
            e1 = span_exp(l, 1, st1, dsum(l, 1))
            span_colsum(l, 0, st0, e0)
            st2b = newst(l, 2)
            span_mm(l, 2, st2b)
            e2 = span_exp(l, 2, st2b, dsum(l, 2))
            span_colsum(l, 1, st1, e1)
            if l + 2 < L:
                preps[l + 2] = prep_tiles(l + 2)
                prep_ssq(l + 2, preps[l + 2], 0, W)
            st3 = newst(l, 3)
            span_mm(l, 3, st3)
            e3 = span_exp(l, 3, st3, dsum(l, 3))
            if l + 2 < L:
                prep_next(l + 2)
            span_colsum(l, 2, st2b, e2)
            span_colsum(l, 3, st3, e3)
            pos_dots(l)
            nc.sync.dma_start(out=out[:, l, :], in_=acc[:, l, :])

        preps = {}
        load(0)
        load(1)
        preps[0] = prep_tiles(0)
        for q0 in range(0, W, QS):
            prep_ssq(0, preps[0], q0, q0 + QS)
            prep_rest(0, preps[0], q0, q0 + QS)
        preps[1] = prep_tiles(1)
        prep_ssq(1, preps[1], 0, W)
        sims0()
        sims(1)
        sims(2)
        sims(3)

    nc.finalize()
    _built = nc
    return nc


def _in_maps(emb_i, emb_j, joint_valid):
    emb_i = np.asarray(emb_i, dtype=np.float32)
    emb_j = np.asarray(emb_j, dtype=np.float32)
    jv = np.asarray(joint_valid, dtype=np.float32).reshape(-1)
    reps = np.concatenate(
        [emb_i.reshape(L, N, D), emb_j.reshape(L, N, D)], axis=1)  # [L, M, D]
    repsc = reps.reshape(L, NCH, 128, D)
    maps = []
    for c in range(NCORES):
        sel = (RC * c + np.arange(W)) % NCH
        win = repsc[:, sel]                       # [L, W, 128, D]
        nat = np.ascontiguousarray(win.transpose(2, 0, 1, 3))  # [128, L, W, D]
        maps.append({"emb_nat": nat})
    return maps, jv


def _combine(results, jv):
    E5 = float(np.exp(INV_T))  # self-similarity exp(1/T)
    denom = np.zeros((L, NCH, 128), dtype=np.float64)
    posg = np.zeros((L, NCH, 128), dtype=np.float64)
    for c, r in enumerate(results):
        a = r["out_all"].astype(np.float64)     # [128, L, 33]
        for l in range(L):
            for i in range(RC):
                g = (RC * c + i) % NCH
                denom[l, g] += a[:, l, i] + a[:, l, 5 + i]
                posg[l, g] = a[:, l, 9 + i]
            denom[l, RC * c % NCH] += a[:, l, 4]  # l0-rc0 split piece B
            for j in range(1, W):
                g = (RC * c + j) % NCH
                denom[l, g] += a[:, l, 13 + j]
    denom -= E5
    w = jv.astype(np.float64)                   # [N]
    wrow = np.concatenate([w, w]).reshape(NCH, 128)  # weight per global row
    lp = -posg * INV_T + np.log(denom)          # [L, NCH, 128]
    loss = (lp * wrow[None]).sum() / (2.0 * w.sum())
    return np.float32(loss)


def kernel(emb_i, emb_j, joint_valid):
    from concourse.bass_utils import run_bass_kernel_spmd

    nc = _build()
    maps, jv = _in_maps(emb_i, emb_j, joint_valid)
    res = run_bass_kernel_spmd(nc, maps, core_ids=list(range(NCORES)))
    return _combine(res.results, jv)


def run_traced(inputs, trace_cores=None):
    """test.py helper: same run but with NTFF tracing enabled."""
    from concourse.bass_utils import run_bass_kernel_spmd

    nc = _build()
    maps, jv = _in_maps(**inputs)
    res = run_bass_kernel_spmd(
        nc, maps, core_ids=list(range(NCORES)), trace=True,
        trace_cores=trace_cores if trace_cores is not None else list(range(NCORES)))
    res.loss = _combine(res.results, jv)
    return res
